# revision 9
# baseline (speedup 1.0000x reference)
"""Trainium2 Bass kernel for nn_DDNWithResidualLoss.

Contract: kernel(**inputs) takes the FULL unsharded inputs (numpy arrays,
keyed as in reference.setup_inputs()) and returns the FULL output (the two
scalar losses). The batch dim B=8 is sharded 1 image per NeuronCore across
8 cores; the box list shards with its image; per-core partial weighted sums
are combined on the host (the cross-device psum is 16 floats).

Architecture (v2, pixel-major, no matmuls):
  The loss is a weighted SUM over pixels, so the host may permute pixels
  freely while sharding. Logits ship PIXEL-MAJOR fp16: x[p, k*81+c] holds
  pixel (p,k)'s 81 channels contiguously. ScalarE streams exp over the
  whole tensor (1 elem/lane/cycle); the softmax denominator s is a
  per-pixel sum over the 81 contiguous channels, computed as a 5-level
  pairwise tensor_tensor ADD tree on DVE in fp16 (2x throughput mode).

  The per-pixel target bin takes <=17 distinct values per image (16 boxes
  + background). The host sorts pixels so each 16-partition x column cell
  is target-homogeneous, which makes the per-pixel channel select a GpSimd
  indirect_copy (per-16-partition-group shared u16 indices, 4-byte chunks:
  gather fp16 PAIRS, keep element 0). The same gather picks the candidate
  residual r_t from a host-gathered 17-row candidate table. Box
  rasterization + LID binning touch only the tiny box inputs and are
  replicated bit-exactly on the host; per-pixel aux (residual target,
  fg/bg weight) ship as fp16 planes. The focal/log epilogue runs on
  DVE/ScalarE over [128, 244] tiles with the final weighted sums fused
  into tensor_tensor_reduce accumulators.
"""

import numpy as np

# ---------------- problem constants (hardcoded per contract) ----------------
B, D, H, W = 8, 80, 96, 320
C = D + 1              # 81 channels
HW = H * W             # 30720 pixels
P = 128                # SBUF partitions
NCAND = 17             # max distinct target bins per image (16 boxes + bg)
NGRP = P // 16         # 8 gpsimd index groups
K = 244                # pixel columns: 8 groups * 244 cells >= 1937 needed
BLOCKS = [(0, 24), (24, 50), (74, 50), (124, 50), (174, 50), (224, 20)]
NEL = K * C            # 19764 elements per partition (x row)
XROW = NEL + 2         # +2 pad for the d=2 gather tail
RROW = K * NCAND       # 4148 (residual candidate row)
RROWP = RROW + 2       # 4150, even, +2 pad for gather tail
SIDX = 16              # wrapped index columns (16*16=256 >= K)
KH0 = 124              # es-gather half 0 columns (blocks 0-2; offsets < 32KB)
KH1 = K - KH0          # 120 (blocks 3-5)
SIDXH = 8              # wrapped index columns per half (8*16=128 >= 124,120)
ALPHA = 0.25
FG_W, BG_W = 13.0, 1.0
DEPTH_MIN, DEPTH_MAX = 0.001, 60.0
N_CORES = 8

f32 = np.float32
f16 = np.float16


# ---------------- host-side reference-exact target computation ----------------
def _host_targets(gt_boxes2d, num_gt_per_img, gt_center_depth):
    """Bit-exact float32 replication of the reference's rasterization+binning.

    Returns per-pixel planes (B, H, W): depth bin target (int32),
    residual target (f32), balancer weight (f32).
    """
    gt_boxes2d = np.asarray(gt_boxes2d, f32)
    gt_center_depth = np.asarray(gt_center_depth, f32)
    num_gt = np.asarray(num_gt_per_img, np.int64)

    u1 = np.floor(gt_boxes2d[:, 0]).astype(np.int32)
    v1 = np.floor(gt_boxes2d[:, 1]).astype(np.int32)
    u2 = np.ceil(gt_boxes2d[:, 2]).astype(np.int32)
    v2 = np.ceil(gt_boxes2d[:, 3]).astype(np.int32)
    ntot = gt_boxes2d.shape[0]

    # jnp.repeat(..., total_repeat_length=ntot): truncate, or pad with the
    # final value (matches jax semantics for the padded tail).
    rep = np.repeat(np.arange(B), np.clip(num_gt, 0, None))
    if len(rep) >= ntot:
        rep = rep[:ntot]
    else:
        pad_val = rep[-1] if len(rep) else 0
        rep = np.concatenate([rep, np.full(ntot - len(rep), pad_val, rep.dtype)])

    dm = np.full((B, H, W), DEPTH_MAX, f32)
    fg = np.zeros((B, H, W), bool)
    for i in range(ntot):
        b = int(rep[i])
        ys = slice(max(int(v1[i]), 0), max(int(v2[i]), 0))
        xs = slice(max(int(u1[i]), 0), max(int(u2[i]), 0))
        dm[b, ys, xs] = np.minimum(dm[b, ys, xs], gt_center_depth[i])
        fg[b, ys, xs] = True

    num_bins = D
    bin_size = f32(2.0 * (DEPTH_MAX - DEPTH_MIN) / (num_bins * (1 + num_bins)))
    with np.errstate(invalid="ignore"):
        idx = f32(-0.5) + f32(0.5) * np.sqrt(
            f32(1.0) + f32(8.0) * (dm - f32(DEPTH_MIN)) / bin_size, dtype=f32
        )
        bad = (idx < 0) | (idx > num_bins) | ~np.isfinite(idx)
        tgt = np.where(bad, num_bins, np.floor(np.where(bad, 0, idx))).astype(np.int32)

    bi = np.arange(num_bins, dtype=f32)
    bin_value = (bi + f32(0.5)) ** 2 * bin_size / f32(2.0) - bin_size / f32(8.0) + f32(DEPTH_MIN)
    bin_values = np.concatenate([bin_value, np.array([DEPTH_MAX], f32)])

    res_tgt = (dm - bin_values[tgt]).astype(f32)
    wgt = np.where(fg, f32(FG_W), f32(BG_W))
    return tgt, res_tgt, wgt


# ---------------- device program ----------------
_PROGRAM = None


def _build_program():
    import concourse.tile as tile
    from concourse import bacc, mybir
    from contextlib import ExitStack

    dt = mybir.dt
    Alu = mybir.AluOpType
    Act = mybir.ActivationFunctionType

    nc = bacc.Bacc("TRN2", target_bir_lowering=False, debug=False)

    x_d = nc.declare_dram_parameter("x", [P, NEL], dt.float16, isOutput=False)
    rc_d = nc.declare_dram_parameter("rc", [P, RROWP], dt.float16, isOutput=False)
    ie0_d = nc.declare_dram_parameter("ie0", [P, SIDXH], dt.uint16, isOutput=False)
    ie1_d = nc.declare_dram_parameter("ie1", [P, SIDXH], dt.uint16, isOutput=False)
    ir_d = nc.declare_dram_parameter("ir", [P, SIDX], dt.uint16, isOutput=False)
    rt_d = nc.declare_dram_parameter("rt", [P, K], dt.float16, isOutput=False)
    w_d = nc.declare_dram_parameter("w", [P, K], dt.float16, isOutput=False)
    out_d = nc.declare_dram_parameter("out", [P, 4], dt.float32, isOutput=True)

    with tile.TileContext(nc) as tc, ExitStack() as ctx:
        main_p = ctx.enter_context(tc.tile_pool(name="main", bufs=1))
        stage_p = ctx.enter_context(tc.tile_pool(name="stage", bufs=1))

        # x block DMAs are issued FIRST so the exp stream starts early;
        # aux tensors are only needed ~15us in.
        xs_tiles = []
        for bi, (k0, kn) in enumerate(BLOCKS):
            xs = stage_p.tile([P, kn * C], dt.float16, tag=f"xs{bi}")
            nc.sync.dma_start(out=xs[:], in_=x_d[:, k0 * C:(k0 + kn) * C])
            xs_tiles.append(xs)
        rc_t = main_p.tile([P, RROWP], dt.float16)
        nc.sync.dma_start(out=rc_t[:], in_=rc_d[:])
        rt_t = main_p.tile([P, K], dt.float16)
        nc.sync.dma_start(out=rt_t[:], in_=rt_d[:])
        w_t = main_p.tile([P, K], dt.float16)
        nc.sync.dma_start(out=w_t[:], in_=w_d[:])
        ie0_t = main_p.tile([P, SIDXH], dt.uint16)
        nc.sync.dma_start(out=ie0_t[:], in_=ie0_d[:])
        ie1_t = main_p.tile([P, SIDXH], dt.uint16)
        nc.sync.dma_start(out=ie1_t[:], in_=ie1_d[:])
        ir_t = main_p.tile([P, SIDX], dt.uint16)
        nc.sync.dma_start(out=ir_t[:], in_=ir_d[:])

        es = main_p.tile([P, XROW], dt.float16)
        nc.gpsimd.memset(es[:, NEL:XROW], 0.0)
        t1 = main_p.tile([P, K, 40], dt.float16)
        t2 = main_p.tile([P, K, 20], dt.float16)
        t3 = main_p.tile([P, K, 10], dt.float16)
        t4 = main_p.tile([P, K, 5], dt.float16)
        t5 = main_p.tile([P, K, 2], dt.float16)
        ua = main_p.tile([P, K], dt.float32)
        ub = main_p.tile([P, K], dt.float32)
        s_t = main_p.tile([P, K], dt.float32)
        et2 = main_p.tile([P, K, 2], dt.float16)
        rs2 = main_p.tile([P, K, 2], dt.float16)

        # rc gather only needs the rc DMA: fire it early
        nc.gpsimd.indirect_copy(
            rs2[:], rc_t[:].rearrange("p (q two) -> p q two", two=2), ir_t[:], True)

        for bi, (k0, kn) in enumerate(BLOCKS):
            cs = slice(k0 * C, (k0 + kn) * C)
            ks = slice(k0, k0 + kn)
            nc.scalar.activation(es[:, cs], xs_tiles[bi][:], Act.Exp)

            ev = es[:, cs].rearrange("p (k c) -> p k c", c=C)
            t1s = t1[:, ks, :]
            t2s = t2[:, ks, :]
            t3s = t3[:, ks, :]
            t4s = t4[:, ks, :]
            t5s = t5[:, ks, :]
            with nc.allow_low_precision("fp16 softmax-denominator tree"):
                nc.vector.tensor_tensor(t1s, ev[:, :, 0:40], ev[:, :, 40:80],
                                        op=Alu.add)
                nc.vector.tensor_tensor(t2s, t1s[:, :, 0:20], t1s[:, :, 20:40],
                                        op=Alu.add)
                nc.vector.tensor_tensor(t3s, t2s[:, :, 0:10], t2s[:, :, 10:20],
                                        op=Alu.add)
                nc.vector.tensor_tensor(t4s, t3s[:, :, 0:5], t3s[:, :, 5:10],
                                        op=Alu.add)
                nc.vector.tensor_tensor(t5s, t4s[:, :, 0:2], t4s[:, :, 2:4],
                                        op=Alu.add)
            nc.vector.tensor_tensor(ua[:, ks], t5s[:, :, 0], t5s[:, :, 1],
                                    op=Alu.add)
            nc.vector.tensor_tensor(ub[:, ks], t4s[:, :, 4], ev[:, :, 80],
                                    op=Alu.add)
            nc.vector.tensor_tensor(s_t[:, ks], ua[:, ks], ub[:, ks],
                                    op=Alu.add)

            if bi == 2:   # blocks 0-2 = columns 0:124 -> es gather half 0
                ev0 = es[:, 0:KH0 * C + 2].rearrange(
                    "p (q two) -> p q two", two=2)
                nc.gpsimd.indirect_copy(et2[:, 0:KH0, :], ev0, ie0_t[:], True)
            if bi == 5:   # blocks 3-5 = columns 124:244 -> half 1 (rebased)
                ev1 = es[:, KH0 * C:XROW].rearrange(
                    "p (q two) -> p q two", two=2)
                nc.gpsimd.indirect_copy(et2[:, KH0:K, :], ev1, ie1_t[:], True)

        # ---- epilogue: per-half where possible; one full-K Ln at the end
        # (avoids Exp<->Ln ACT table thrash mid-stream). alpha and
        # /num_pixels are folded on the host.
        rec = main_p.tile([P, K], dt.float32)
        pt = main_p.tile([P, K], dt.float32)
        u = main_p.tile([P, K], dt.float32)
        focal = main_p.tile([P, K], dt.float32)
        fw = main_p.tile([P, K], dt.float32)
        dres = main_p.tile([P, K], dt.float32)
        ndres = main_p.tile([P, K], dt.float32)
        ares = main_p.tile([P, K], dt.float32)
        scr2 = main_p.tile([P, K], dt.float32)
        lnp = main_p.tile([P, K], dt.float32)
        lw = main_p.tile([P, K], dt.float32)
        scr = main_p.tile([P, K], dt.float32)
        part = main_p.tile([P, 4], dt.float32)

        halves = [slice(0, KH0), slice(KH0, K)]
        for h, hs in enumerate(halves):
            nc.vector.reciprocal(rec[:, hs], s_t[:, hs])
            nc.vector.tensor_tensor(pt[:, hs], et2[:, hs, 0], rec[:, hs],
                                    op=Alu.mult)
            nc.vector.tensor_scalar(u[:, hs], pt[:, hs], -1.0, 1.0,
                                    op0=Alu.mult, op1=Alu.add)
            nc.vector.tensor_tensor(focal[:, hs], u[:, hs], u[:, hs],
                                    op=Alu.mult)
            nc.vector.tensor_tensor(fw[:, hs], focal[:, hs], w_t[:, hs],
                                    op=Alu.mult)
            nc.vector.tensor_tensor(dres[:, hs], rs2[:, hs, 0], rt_t[:, hs],
                                    op=Alu.subtract)
            nc.vector.tensor_scalar(ndres[:, hs], dres[:, hs], -1.0, None,
                                    op0=Alu.mult)
            nc.vector.tensor_tensor(ares[:, hs], dres[:, hs], ndres[:, hs],
                                    op=Alu.max)
            nc.vector.tensor_tensor(scr2[:, hs], ares[:, hs], fw[:, hs],
                                    op=Alu.mult)
            nc.vector.tensor_reduce(part[:, 2 + h:3 + h], scr2[:, hs],
                                    axis=mybir.AxisListType.X, op=Alu.add)

        nc.scalar.activation(lnp[:], pt[:], Act.Ln)
        for h, hs in enumerate(halves):
            nc.vector.tensor_tensor(lw[:, hs], lnp[:, hs], w_t[:, hs],
                                    op=Alu.mult)
            nc.vector.tensor_tensor(scr[:, hs], focal[:, hs], lw[:, hs],
                                    op=Alu.mult)
            nc.vector.tensor_reduce(part[:, h:1 + h], scr[:, hs],
                                    axis=mybir.AxisListType.X, op=Alu.add)
        nc.sync.dma_start(out=out_d[:], in_=part[:])

    nc.compile()
    return nc


def _get_program():
    global _PROGRAM
    if _PROGRAM is None:
        _PROGRAM = _build_program()
    return _PROGRAM


LAST_RESULTS = None  # populated with the BassKernelResults of the last run


def _wrap_idx(lin_idx, scols):
    """(NGRP, n) linear gather indices -> [P, scols] u16 wrapped layout."""
    n = lin_idx.shape[1]
    out = np.zeros((P, scols), np.uint16)
    k = np.arange(n)
    for g in range(NGRP):
        out[16 * g + (k % 16), k // 16] = lin_idx[g]
    return out


def _build_in_maps(depth_logits, depth_residuals, tgt, res_tgt, wgt):
    """depth_logits/depth_residuals: (B, C, HW); tgt/res_tgt/wgt: (B, ...)."""
    in_maps = []
    ncells_grid = NGRP * K
    for b in range(N_CORES):
        tgt_b = tgt[b].reshape(HW)
        c_list = np.unique(tgt_b)
        assert len(c_list) <= NCAND, f"more than {NCAND} distinct bins"
        j_pix = np.searchsorted(c_list, tgt_b)

        # group pixels by target position j into 16-pixel cells (-1 pads)
        cell_rows = []
        cell_js = []
        for j in range(len(c_list)):
            pix = np.flatnonzero(j_pix == j)
            ncell = -(-len(pix) // 16)
            pad = ncell * 16 - len(pix)
            if pad:
                pix = np.concatenate([pix, np.full(pad, -1, pix.dtype)])
            cell_rows.append(pix.reshape(ncell, 16))
            cell_js.append(np.full(ncell, j, np.int64))
        cells = np.concatenate(cell_rows)
        cj = np.concatenate(cell_js)
        assert len(cells) <= ncells_grid, f"{len(cells)} cells > {ncells_grid}"
        padc = ncells_grid - len(cells)
        if padc:
            cells = np.concatenate(
                [cells, np.full((padc, 16), -1, cells.dtype)])
            cj = np.concatenate([cj, np.zeros(padc, cj.dtype)])

        # cell m -> (group g = m // K, column k = m % K); slot partition
        # p = 16*g + q holds pixel cells[m, q]
        perm = cells.reshape(NGRP, K, 16).transpose(0, 2, 1).reshape(P, K)
        valid = perm >= 0
        slot = np.where(valid, perm, 0)

        xT = depth_logits[b].reshape(C, HW).T          # [HW, 81]
        x_pm = xT[slot].astype(f16)                    # [P, K, 81]

        r17 = depth_residuals[b].reshape(C, HW)[
            np.concatenate([c_list,
                            np.full(NCAND - len(c_list), c_list[0],
                                    c_list.dtype)])]    # [17, HW]
        rc_pm = np.zeros((P, RROWP), f16)
        rc_pm[:, :RROW] = r17.T[slot].astype(f16).reshape(P, RROW)

        rt_pm = np.where(valid, res_tgt[b].reshape(HW)[slot], 0).astype(f16)
        w_pm = np.where(valid, wgt[b].reshape(HW)[slot], 0).astype(f16)

        cjk = cj.reshape(NGRP, K)                      # per (group, col) j
        ck = np.arange(K)[None, :]
        ie = ck * C + c_list[cjk]                      # es gather index
        ir = (ck * NCAND + cjk).astype(np.uint16)      # rcand gather index
        ie0 = ie[:, :KH0].astype(np.uint16)            # half 0: data base 0
        ie1 = (ie[:, KH0:] - KH0 * C).astype(np.uint16)  # half 1: rebased

        in_maps.append({
            "x": np.ascontiguousarray(x_pm.reshape(P, NEL)),
            "rc": rc_pm,
            "ie0": _wrap_idx(ie0, SIDXH),
            "ie1": _wrap_idx(ie1, SIDXH),
            "ir": _wrap_idx(ir, SIDX),
            "rt": rt_pm,
            "w": w_pm,
        })
    return in_maps


def kernel(depth_logits, depth_residuals, gt_boxes2d, num_gt_per_img, gt_center_depth):
    global LAST_RESULTS
    from concourse.bass_utils import run_bass_kernel_spmd

    depth_logits = np.ascontiguousarray(np.asarray(depth_logits, f32))
    depth_residuals = np.ascontiguousarray(np.asarray(depth_residuals, f32))

    tgt, res_tgt, wgt = _host_targets(gt_boxes2d, num_gt_per_img, gt_center_depth)
    in_maps = _build_in_maps(depth_logits.reshape(B, C, HW),
                             depth_residuals.reshape(B, C, HW),
                             tgt, res_tgt, wgt)

    nc = _get_program()
    res = run_bass_kernel_spmd(nc, in_maps, list(range(N_CORES)))
    LAST_RESULTS = res

    acc = np.zeros(4, np.float64)
    for b in range(N_CORES):
        acc += np.asarray(res.results[b]["out"], np.float64).sum(axis=0)
    num_pixels = float(B * H * W)
    map_loss = f32(-ALPHA * (acc[0] + acc[1]) / num_pixels)
    res_loss = f32(ALPHA * (acc[2] + acc[3]) / num_pixels)
    return map_loss, res_loss
